# revision 60
# baseline (speedup 1.0000x reference)
"""Trainium2 Bass kernel for an attention block (MHSA with relative position
bias + 2x LayerNorm + FFN), sharded over 8 NeuronCores.

Sharding: tensor-parallel over heads for attention (core c owns head c, both
batch elements). Heads are exchanged with a bf16 AllToAll (per-head outputs,
already normalized, transposed [feat, tok]) so each core then computes the
out-projection, residual+LN1, FFN and LN2 for its own 512-token shard
locally. The host reassembles the full [2, 2048, 512] output.

v2 layout choices (vs the f32r baseline):
  - All matmul operands are bf16 (psum stays f32); weights and x are
    converted host-side, halving HBM traffic.
  - 1/sqrt(dh) is folded into Wq/bq host-side, and the relative-position
    bias is applied as exp(s)*exp(b/8): the exp runs on ACT straight from
    the scores psum, and the Toeplitz exp-band multiply runs on DVE in
    2-byte fast mode. No f32 prescale pass.
  - The exp-band is materialized once in SBUF as a [128, 4096] sliding
    window (row j' = table[j':j'+4096]); every (jt,b) tile is a reversed
    in-SBUF slice - no per-tile band DMA.
  - Softmax 1/sigma is broadcast across the 64 feature partitions with a
    rank-1 PE matmul (ones[1,64]^T @ rrow[1,1024]) instead of a DRAM
    bounce, keeping the pre-AllToAll path on-chip.
  - Phase C runs ihalf0's FFN completely (through LN2) before touching
    ihalf1, so the second AllToAll is covered by useful work.
"""
import os
import sys

for _p in ("/opt/trn_rl_repo", "/root/.axon_site/_ro/trn_rl_repo"):
    if os.path.isdir(_p) and _p not in sys.path:
        sys.path.insert(0, _p)

import numpy as np
import ml_dtypes

import concourse.bass as bass
import concourse.mybir as mybir
import concourse.tile as tile
from concourse import bacc
from concourse import bass_utils

F32 = mybir.dt.float32
F32R = mybir.dt.float32r
BF16 = mybir.dt.bfloat16
AF = mybir.ActivationFunctionType
ALU = mybir.AluOpType

N_CORES = 8
B, L, D, DFF = 2, 2048, 512, 2048
H, DH = 8, 64
L_MAX = 39000
LN_EPS = 1e-5
NT = B * L               # 4096 tokens
P = 128
IH = 1024                # i-half size inside one batch's 2048 queries
NJT = L // P             # 16 j tiles per batch
SCALE = 0.125            # 1/sqrt(dh), folded into Wq host-side
VW = DH + 32             # v block width: 64 v cols + [ones | 31 zeros]
#                          (the sigma block is 32 wide so the DoubleRow
#                           stationary stays 32-col aligned)
EBW = 4352               # exp-band table length (>= 128 + 4095)
EXPC = 4.0               # exp shift: pt = exp(s/8 - EXPC) keeps the fp8
#                          weights in e4m3 range (softmax is shift-invariant)
FP8 = mybir.dt.float8e4

_cached = {}


def _ln_pair(nc, scr, h_aps, out_aps, eps_ap):
    """LayerNorm (without the affine epilogue - gamma/beta are folded on
    the host) over the free dim (512) for a PAIR of [128, 512] tiles,
    stage-batched so ACT table loads and cross-engine hops amortize."""
    n = len(h_aps)
    sq = [scr.tile([P, D], F32, tag=f"ln_sq{k}", name=f"lnsq{k}")
          for k in range(n)]
    ssum = [scr.tile([P, 1], F32, tag=f"ln_s1{k}", name=f"lns1{k}")
            for k in range(n)]
    msum = [scr.tile([P, 1], F32, tag=f"ln_s2{k}", name=f"lns2{k}")
            for k in range(n)]
    mu = [scr.tile([P, 1], F32, tag=f"ln_s3{k}", name=f"lns3{k}")
          for k in range(n)]
    var = [scr.tile([P, 1], F32, tag=f"ln_s5{k}", name=f"lns5{k}")
           for k in range(n)]
    std = [scr.tile([P, 1], F32, tag=f"ln_s6{k}", name=f"lns6{k}")
           for k in range(n)]
    rstd = [scr.tile([P, 1], F32, tag=f"ln_s7{k}", name=f"lns7{k}")
            for k in range(n)]
    nmr = [scr.tile([P, 1], F32, tag=f"ln_s8{k}", name=f"lns8{k}")
           for k in range(n)]
    # DVE-heavy: only the Sqrt crosses to ACT (2 engine hops total)
    for k in range(n):
        nc.vector.scalar_tensor_tensor(out=sq[k][:], in0=h_aps[k],
                                       scalar=1.0, in1=h_aps[k],
                                       op0=ALU.mult, op1=ALU.mult,
                                       accum_out=ssum[k][:])
    for k in range(n):
        nc.vector.tensor_reduce(out=msum[k][:], in_=h_aps[k],
                                axis=mybir.AxisListType.X, op=ALU.add)
    for k in range(n):
        nc.vector.tensor_scalar_mul(mu[k][:], msum[k][:], 1.0 / D)
        nc.vector.tensor_scalar_mul(var[k][:], ssum[k][:], 1.0 / D)
    for k in range(n):
        nc.vector.tensor_mul(msum[k][:], mu[k][:], mu[k][:])
        nc.vector.tensor_sub(var[k][:], var[k][:], msum[k][:])
    for k in range(n):
        nc.scalar.activation(std[k][:], var[k][:], AF.Sqrt, bias=eps_ap)
    for k in range(n):
        nc.vector.reciprocal(rstd[k][:], std[k][:])
        nc.vector.scalar_tensor_tensor(out=nmr[k][:], in0=mu[k][:],
                                       scalar=-1.0, in1=rstd[k][:],
                                       op0=ALU.mult, op1=ALU.mult)
    for k in range(n):
        nc.vector.tensor_scalar(out=out_aps[k], in0=h_aps[k],
                                scalar1=rstd[k][:], scalar2=nmr[k][:],
                                op0=ALU.mult, op1=ALU.add)


def build():
    nc = bacc.Bacc("TRN2", target_bir_lowering=False, debug=False,
                   num_devices=N_CORES)

    # ---- I/O ----
    xT = nc.dram_tensor("xT", [D, NT], BF16, kind="ExternalInput")
    xsh = nc.dram_tensor("xsh", [4 * P, D], F32, kind="ExternalInput")
    wqk = nc.dram_tensor("wqk", [D, 2 * DH], BF16, kind="ExternalInput")
    wv = nc.dram_tensor("wv", [D, DH], BF16, kind="ExternalInput")
    wo = nc.dram_tensor("wo", [D, D], BF16, kind="ExternalInput")
    w1 = nc.dram_tensor("w1", [D, DFF], BF16, kind="ExternalInput")
    w2 = nc.dram_tensor("w2", [DFF, D], BF16, kind="ExternalInput")
    eband = nc.dram_tensor("eband", [EBW], BF16, kind="ExternalInput")
    bqk = nc.dram_tensor("bqk", [2 * DH, 1], F32, kind="ExternalInput")
    bv = nc.dram_tensor("bv", [DH, 1], F32, kind="ExternalInput")
    b1c = nc.dram_tensor("b1c", [P, DFF // P], F32, kind="ExternalInput")
    bo = nc.dram_tensor("bo", [D], F32, kind="ExternalInput")
    b2 = nc.dram_tensor("b2", [D], F32, kind="ExternalInput")
    g1 = nc.dram_tensor("g1", [D], BF16, kind="ExternalInput")
    out_sh = nc.dram_tensor("out_sh", [4 * P, D], F32, kind="ExternalOutput")

    with tile.TileContext(nc) as tc:
        with tc.tile_pool(name="persist", bufs=1) as pers, \
             tc.tile_pool(name="phC_w", bufs=1) as pCw, \
             tc.tile_pool(name="dram", bufs=1, space="DRAM") as dram:

            # ---------- persistent SBUF (small) ----------
            ident32 = pers.tile([P, P], F32)
            identb = pers.tile([P, P], BF16)
            ident8 = pers.tile([P, P], FP8)
            bqk_sb = pers.tile([2 * DH, 1], F32)
            bv_sb = pers.tile([DH, 1], F32)
            wo_sb = [pers.tile([P, D], BF16, name=f"wo{c}") for c in range(4)]
            eps_sb = pers.tile([P, 1], F32)
            nc.vector.memset(eps_sb[:], LN_EPS)
            negc_sb = pers.tile([P, 1], F32)
            nc.vector.memset(negc_sb[:], -EXPC)
            ones_row = pers.tile([1, DH], BF16)
            nc.vector.memset(ones_row[:], 1.0)
            warm_sb = pers.tile([1, 1], F32)
            # one a2a per 128-token tile, pipelined with attention: only
            # tile 3's FFN chain remains after the last (small) collective
            oaq_sb = [[pers.tile([P, P], BF16, name=f"oa{q}_{c}")
                       for c in range(4)] for q in range(4)]
            hh0_sb = [pers.tile([P, D], F32, name=f"hh0_{m}")
                      for m in range(3)]
            x_sb = [[pers.tile([P, D], F32, name=f"x_{ih}_{b_}")
                     for b_ in range(B)] for ih in range(2)]

            from concourse.masks import make_identity
            make_identity(nc, ident32[:])
            nc.scalar.copy(identb[:], ident32[:])
            nc.scalar.copy(ident8[:], ident32[:])

            nc.sync.dma_start(bqk_sb[:], bqk.ap())
            nc.sync.dma_start(bv_sb[:], bv.ap())

            # a2a buffers: one [8*64, 128] exchange per token tile
            a2a_in = [dram.tile([N_CORES * DH, P], BF16, name=f"a2ai{i}")
                      for i in range(4)]
            a2a_out = [dram.tile([N_CORES * DH, P], BF16, name=f"a2ao{i}")
                       for i in range(4)]

            # ================= attention (phases A+B) =================
            with tc.tile_pool(name="attn_sb", bufs=1) as patt:
                # q/k in the canonical fp8 DoubleRow layout [128, 2, tok]:
                # subtile 0 rows 0..63 hold the head dim, the rest is zero
                # (matmul cost scales with output columns, not K rows)
                q8 = patt.tile([P, 2, NT], FP8)
                k8 = patt.tile([P, 2, NT], FP8)
                v_aug = patt.tile([P, 32, VW], FP8)
                outT_sb = [patt.tile([DH, L], BF16, name=f"outT{b_}")
                           for b_ in range(B)]
                # sliding-window exp-band: row j' = eband[j' : j'+4096]
                eb_all = patt.tile([P, 4096], BF16)

                # ---------- phase A: qkv projections ----------
                with tc.tile_pool(name="phA", bufs=1) as pA, \
                     tc.tile_pool(name="psA", bufs=2, space="PSUM") as psA:
                    wqk_sb = [pA.tile([P, 2 * DH], BF16, name=f"wqk{c}")
                              for c in range(4)]
                    wv_sb = [pA.tile([P, DH], BF16, name=f"wv{c}")
                             for c in range(4)]
                    for c in range(4):
                        nc.sync.dma_start(wqk_sb[c][:],
                                          wqk.ap()[c * P:(c + 1) * P, :])
                        nc.sync.dma_start(wv_sb[c][:],
                                          wv.ap()[c * P:(c + 1) * P, :])

                    vT_sb = pA.tile([DH, NT], BF16)
                    # x chunks split over the sync+ACT DMA queues; the
                    # gpsimd queue carries the no-dependency loads so the
                    # dependent kT copies never delay an x transfer
                    qdma = [nc.sync, nc.sync, nc.scalar, nc.scalar]
                    for hf in range(2):
                        xh = [pA.tile([P, L], BF16, tag=f"xh{c}",
                                      name=f"xh_{hf}_{c}", bufs=2)
                              for c in range(4)]
                        for c in range(4):
                            qdma[c].dma_start(
                                xh[c][:],
                                xT.ap()[c * P:(c + 1) * P,
                                        hf * L:(hf + 1) * L])
                        if hf == 0:
                            nc.gpsimd.dma_start(
                                eb_all[:],
                                bass.AP(eband, 0, [[1, P], [1, 4096]]))
                            # zero the unused DoubleRow K rows/subtiles
                            nc.vector.memset(q8[DH:, 0, :], 0.0)
                            nc.vector.memset(q8[:, 1, :], 0.0)
                            nc.gpsimd.memset(k8[DH:, 0, :], 0.0)
                            nc.gpsimd.memset(k8[:, 1, :], 0.0)
                            for t in range(32):
                                nc.vector.memset(v_aug[:, t, DH:VW], 0.0)
                                nc.vector.memset(
                                    v_aug[:, t, DH:DH + 1], 1.0)
                            for ih in range(2):
                                for b_ in range(B):
                                    ci = b_ * 2 + ih
                                    nc.gpsimd.dma_start(
                                        x_sb[ih][b_][:],
                                        xsh.ap()[ci * P:(ci + 1) * P, :])
                        def v_transposes(t):
                            # transpose v^T -> v_aug natural [tok, dh];
                            # bf16 through the PE (fp8 transpose has an
                            # output-step constraint), cast to fp8 on the
                            # psum->SBUF copy. Ones cols were memset above.
                            for tt in range(4 * t, 4 * t + 4):
                                ps2 = psA.tile([P, DH], BF16, tag="vt_ps")
                                nc.tensor.transpose(
                                    ps2[:], vT_sb[:, tt * P:(tt + 1) * P],
                                    identb[:DH, :DH])
                                nc.scalar.copy(
                                    v_aug[:, tt, :DH], ps2[:])

                        for t4 in range(4):
                            t = hf * 4 + t4
                            sl = slice(t * 512, (t + 1) * 512)
                            ps = psA.tile([P, 512], F32, tag="qk_ps")
                            for c in range(4):
                                nc.tensor.matmul(
                                    ps[:], wqk_sb[c][:],
                                    xh[c][:, t4 * 512:(t4 + 1) * 512],
                                    start=(c == 0), stop=(c == 3))
                            # q|k evac on DVE (beside the v evac on ACT)
                            # into DoubleRow subtile 0, rows 0..63
                            nc.vector.tensor_scalar(
                                out=q8[:DH, 0, sl], in0=ps[:DH, :],
                                scalar1=bqk_sb[:DH, :],
                                scalar2=None, op0=ALU.add)
                            nc.vector.tensor_scalar(
                                out=k8[:DH, 0, sl], in0=ps[DH:, :],
                                scalar1=bqk_sb[DH:, :],
                                scalar2=None, op0=ALU.add)
                            psv = psA.tile([DH, 512], F32, tag="v_ps")
                            for c in range(4):
                                nc.tensor.matmul(
                                    psv[:], wv_sb[c][:],
                                    xh[c][:, t4 * 512:(t4 + 1) * 512],
                                    start=(c == 0), stop=(c == 3))
                            nc.scalar.activation(
                                vT_sb[:, t * 512:(t + 1) * 512], psv[:],
                                AF.Identity, bias=bv_sb[:])
                            # pipeline the v transposes one tile behind so
                            # they never stall the projection matmuls
                            if t > 0:
                                v_transposes(t - 1)
                        if hf == 1:
                            v_transposes(7)

                # prefetch out-proj + FFN weights now that phase A freed
                # SBUF; the sync DMA queue is otherwise idle here, and the
                # ACT queue must stay clear for the attention exps
                for c in range(4):
                    nc.sync.dma_start(wo_sb[c][:],
                                      wo.ap()[c * P:(c + 1) * P, :])
                w1_sb = [pCw.tile([P, DFF], BF16, name=f"w1_{c}")
                         for c in range(4)]
                for c in range(4):
                    nc.sync.dma_start(w1_sb[c][:],
                                      w1.ap()[c * P:(c + 1) * P, :])
                w2_sb = [pCw.tile([P, D], BF16, name=f"w2_{q}")
                         for q in range(16)]
                for q in range(16):
                    nc.sync.dma_start(w2_sb[q][:],
                                      w2.ap()[q * P:(q + 1) * P, :])
                b1_sb = pCw.tile([P, DFF // P], F32)
                nc.sync.dma_start(b1_sb[:], b1c.ap())
                reps = {}
                for nm, t, dt in (("bo", bo, F32), ("b2", b2, F32),
                                  ("g1", g1, BF16)):
                    r = pCw.tile([P, D], dt, name=f"rep_{nm}")
                    nc.gpsimd.dma_start(
                        r[:], t.ap().unsqueeze(0).broadcast_to([P, D]))
                    reps[nm] = r

                # ---------- phase B: attention, batch-outer ----------
                with tc.tile_pool(name="phB_p", bufs=4) as pP, \
                     tc.tile_pool(name="phB_r", bufs=2) as pR, \
                     tc.tile_pool(name="psB_s", bufs=3, space="PSUM") as psS, \
                     tc.tile_pool(name="psB_o", bufs=1, space="PSUM") as psO:
                    DR = mybir.MatmulPerfMode.DoubleRow
                    for b_ in range(B):
                        tb = b_ * L
                        for ihalf in range(2):
                            i0g = ihalf * IH
                            oT = psO.tile([VW, IH], F32, tag="outT_ps",
                                          name=f"oT_{b_}_{ihalf}")
                            ppts = {}

                            def do_av(pr):
                                # fp8 DoubleRow AV: one matmul contracts the
                                # j-tile PAIR (2*128 keys); output cols are
                                # [64 v | ones(sigma) | 31 zero] = M=96 so
                                # the psum write stays at partition base 0
                                t0 = b_ * NJT + 2 * pr
                                ppt = ppts.pop(pr)
                                for n2 in range(2):
                                    nsl = slice(n2 * 512, (n2 + 1) * 512)
                                    nc.tensor.matmul(
                                        oT[:, nsl],
                                        v_aug[:, t0:t0 + 2, :],
                                        ppt[:, :, nsl],
                                        start=(pr == 0),
                                        stop=(pr == NJT // 2 - 1),
                                        perf_mode=DR)

                            for jt in range(NJT):
                                pr = jt // 2
                                # band tile: row j', col i -> eb[q0+j'+IH-1-i]
                                q0 = jt * P + IH * (1 - ihalf)
                                wrev = eb_all[:, q0 + IH - 1::-1][:, :IH]
                                sps = psS.tile([P, IH], F32, tag="s_ps")
                                for n2 in range(2):
                                    nc.tensor.matmul(
                                        sps[:, n2 * 512:(n2 + 1) * 512],
                                        k8[:, :, tb + jt * P:
                                           tb + (jt + 1) * P],
                                        q8[:, :, tb + i0g + n2 * 512:
                                           tb + i0g + (n2 + 1) * 512],
                                        start=True, stop=True,
                                        perf_mode=DR)
                                es = pP.tile([P, IH], BF16, tag="es")
                                nc.scalar.activation(es[:], sps[:], AF.Exp,
                                                     scale=SCALE,
                                                     bias=negc_sb[:])
                                if jt % 2 == 0:
                                    ppt_new = pP.tile([P, 2, IH], FP8,
                                                      tag="ppt", bufs=3,
                                                      name="ppt")
                                    ppts[pr] = ppt_new
                                # band multiply -> fp8 weights; split over
                                # DVE and the otherwise-idle Pool engine
                                meng = nc.gpsimd if jt % 3 == 2 else nc.vector
                                meng.tensor_mul(ppts[pr][:, jt % 2, :],
                                                es[:], wrev)
                                if jt % 2 == 1 and pr >= 1:
                                    do_av(pr - 1)
                            do_av(NJT // 2 - 1)
                            if b_ == 1 and ihalf == 1:
                                # prewarm ACT Sqrt table so LN1 doesn't pay
                                # the table reload on the post-attention path
                                nc.scalar.activation(warm_sb[:],
                                                     eps_sb[:1, :1],
                                                     AF.Sqrt, bias=0.0)
                            # evict: normalize by 1/sigma; sigma row is
                            # broadcast over the 64 feature partitions with
                            # a rank-1 PE matmul (no DRAM bounce)
                            sl = slice(i0g, i0g + IH)
                            srow = pR.tile([1, IH], F32, tag="srow")
                            nc.vector.tensor_copy(srow[:],
                                                  oT[DH:DH + 1, :])
                            rrow = pR.tile([1, IH], F32, tag="rrow")
                            rscr = pR.tile([1, IH], F32, tag="rscr")
                            nc.vector.reciprocal_approx_accurate(
                                rrow[:], srow[:], rscr[:])
                            rrowb = pR.tile([1, IH], BF16, tag="rrowb")
                            nc.vector.tensor_copy(rrowb[:], rrow[:])
                            rrep = psS.tile([DH, IH], F32, tag="s_ps",
                                            name=f"rrep_{b_}_{ihalf}")
                            for n2 in range(2):
                                nc.tensor.matmul(
                                    rrep[:, n2 * 512:(n2 + 1) * 512],
                                    ones_row[:],
                                    rrowb[:, n2 * 512:(n2 + 1) * 512],
                                    start=True, stop=True)
                            rreps = pR.tile([DH, IH], F32, tag="rreps")
                            nc.vector.tensor_copy(rreps[:], rrep[:])
                            nc.vector.tensor_mul(outT_sb[b_][:, sl],
                                                 oT[:DH, :], rreps[:])
                            # ship this chunk's pieces and exchange heads
                            # immediately (q3 = token-tile index b*2+ihalf);
                            # sync+gpsimd queues: ACT stays clear for exps
                            q3 = b_ * 2 + ihalf
                            for cd in range(N_CORES):
                                eng = nc.sync if cd % 2 == 0 else nc.gpsimd
                                eng.dma_start(
                                    a2a_in[q3][cd * DH:(cd + 1) * DH, :],
                                    outT_sb[b_][:, i0g + cd * P:
                                                i0g + (cd + 1) * P])
                            nc.gpsimd.collective_compute(
                                "AllToAll", ALU.bypass,
                                replica_groups=[list(range(N_CORES))],
                                ins=[a2a_in[q3][:].opt()],
                                outs=[a2a_out[q3][:].opt()])
                            for c in range(4):
                                nc.gpsimd.dma_start(
                                    oaq_sb[q3][c][:],
                                    a2a_out[q3][c * P:(c + 1) * P, :])

                    # hoisted: out-proj + residual for a2a#1's three token
                    # tiles runs in the attention psum pool (s_ps tag) right
                    # at attention end, avoiding cross-pool bank-alias waits
                    for m in range(3):
                        pop0 = psS.tile([P, D], F32, tag="s_ps",
                                        name=f"pop0_{m}")
                        for c in range(4):
                            nc.tensor.matmul(
                                pop0[:, :D],
                                oaq_sb[m][c][:],
                                wo_sb[c][:], start=(c == 0), stop=(c == 3))
                        nc.vector.scalar_tensor_tensor(
                            out=hh0_sb[m][:], in0=pop0[:, :D], scalar=1.0,
                            in1=x_sb[m % 2][m // 2][:], op0=ALU.mult,
                            op1=ALU.add)
                        nc.vector.tensor_add(hh0_sb[m][:], hh0_sb[m][:],
                                             reps["bo"][:])

            # ---------- phase C: out-proj + residual + LN1 + FFN + LN2 ----
            # batch0's full chain runs first so it covers the a2a#2 wait
            with tc.tile_pool(name="phC", bufs=2) as pC, \
                 tc.tile_pool(name="phC_scr", bufs=2) as scr, \
                 tc.tile_pool(name="psC", bufs=2, space="PSUM") as psC, \
                 tc.tile_pool(name="psC2", bufs=2, space="PSUM") as psC2, \
                 tc.tile_pool(name="psC3", bufs=2, space="PSUM") as psC3:
                # group 0 = token tiles 0..2 (a2a#1), group 1 = tile 3 (#2)
                GT = {0: [0, 1, 2], 1: [3]}
                hgs = {}
                hgTs = {}
                f1Ts = {}

                def prep(g):
                    tl = GT[g]
                    # LN1 output directly in bf16 (matmul + residual path)
                    hgb = [pC.tile([P, D], BF16, tag="hgb", bufs=5,
                                   name=f"hgb_{ci}") for ci in tl]
                    hgs[g] = hgb
                    if g == 0:
                        hhs = hh0_sb
                    else:
                        hhs = []
                        for ci in tl:
                            pop = psC3.tile([P, D], F32, tag="po_ps")
                            for c in range(4):
                                nc.tensor.matmul(
                                    pop[:], oaq_sb[ci][c][:],
                                    wo_sb[c][:], start=(c == 0),
                                    stop=(c == 3))
                            hh = pC.tile([P, D], F32, tag="hh", bufs=4,
                                         name=f"hh_{ci}")
                            nc.vector.scalar_tensor_tensor(
                                out=hh[:], in0=pop[:], scalar=1.0,
                                in1=x_sb[ci % 2][ci // 2][:], op0=ALU.mult,
                                op1=ALU.add)
                            nc.vector.tensor_add(hh[:], hh[:],
                                                 reps["bo"][:])
                            hhs.append(hh)
                    _ln_pair(nc, scr, [t[:] for t in hhs],
                             [t[:] for t in hgb], eps_sb[:])

                def transposes(g):
                    tl = GT[g]
                    W = len(tl) * P
                    hgT = pC.tile([P, 4 * W], BF16, tag=f"hgT{g}", bufs=1,
                                  name=f"hgT_{g}")
                    hgTs[g] = (hgT, W)
                    for m in range(len(tl)):
                        for c in range(4):
                            tps = psC.tile([P, P], BF16, tag="tr_ps")
                            nc.tensor.transpose(
                                tps[:], hgs[g][m][:, c * P:(c + 1) * P],
                                identb[:])
                            nc.scalar.copy(
                                hgT[:, c * W + m * P:c * W + (m + 1) * P],
                                tps[:])

                def ffn1(g):
                    hgT, W = hgTs[g]
                    f1T = pC.tile([P, 16 * W], BF16, tag=f"f1T{g}", bufs=1,
                                  name=f"f1T_{g}")
                    f1Ts[g] = (f1T, W)
                    # batch TB dff-tiles per psum tile so narrow-W groups
                    # don't pay per-matmul/relu overheads 16 times
                    TB = max(1, 512 // W)
                    for t0 in range(0, 16, TB):
                        fps = psC.tile([P, TB * W], F32, tag="f1_ps")
                        for tq in range(TB):
                            t = t0 + tq
                            for c in range(4):
                                nc.tensor.matmul(
                                    fps[:, tq * W:(tq + 1) * W],
                                    w1_sb[c][:, t * P:(t + 1) * P],
                                    hgT[:, c * W:(c + 1) * W],
                                    start=(c == 0), stop=(c == 3))
                        if TB == 1:
                            nc.scalar.activation(
                                f1T[:, t0 * W:(t0 + TB) * W], fps[:],
                                AF.Relu, bias=b1_sb[:, t0:t0 + 1])
                        else:
                            # b1 varies per dff-tile: bias via per-tile AP
                            for tq in range(TB):
                                t = t0 + tq
                                nc.scalar.activation(
                                    f1T[:, t * W:(t + 1) * W],
                                    fps[:, tq * W:(tq + 1) * W],
                                    AF.Relu, bias=b1_sb[:, t:t + 1])

                def ffn2_out(g):
                    tl = GT[g]
                    f1T, W = f1Ts[g]
                    zzs = []
                    for m, ci in enumerate(tl):
                        ops = psC2.tile([P, D], F32, tag="f2_ps")
                        for q in range(16):
                            nc.tensor.matmul(
                                ops[:],
                                f1T[:, q * W + m * P:q * W + (m + 1) * P],
                                w2_sb[q][:], start=(q == 0), stop=(q == 15))
                        # LN2 residual: hnorm1*g1 (be1 is folded into b2)
                        hgr = pC.tile([P, D], BF16, tag="hgr", bufs=4,
                                      name=f"hgr_{ci}")
                        nc.vector.tensor_mul(hgr[:], hgs[g][m][:],
                                             reps["g1"][:])
                        zz = pC.tile([P, D], F32, tag="zz", bufs=4,
                                     name=f"zz_{ci}")
                        nc.vector.tensor_add(zz[:], ops[:], hgr[:])
                        nc.vector.tensor_add(zz[:], zz[:], reps["b2"][:])
                        zzs.append(zz)
                    yys = [pC.tile([P, D], F32, tag="yy", bufs=4,
                                   name=f"yy_{ci}") for ci in tl]
                    _ln_pair(nc, scr, [t[:] for t in zzs],
                             [t[:] for t in yys], eps_sb[:])
                    for m, ci in enumerate(tl):
                        nc.sync.dma_start(out_sh.ap()[ci * P:(ci + 1) * P, :],
                                          yys[m][:])

                prep(0)
                transposes(0)
                ffn1(0)
                ffn2_out(0)
                prep(1)
                transposes(1)
                ffn1(1)
                ffn2_out(1)
    nc.compile()
    return nc


def _prep_inputs(x, Wqkv, bqkv, Wo, bo, pos_bias, W1, b1, W2, b2,
                 gamma1, beta1, gamma2, beta2):
    x_flat = np.ascontiguousarray(x.reshape(NT, D), dtype=np.float32)
    xT = np.ascontiguousarray(x_flat.T).astype(ml_dtypes.bfloat16)
    wo_full = np.asarray(Wo, np.float32).astype(ml_dtypes.bfloat16)
    # LN affine folds: y = LN(h)*g + b with LN the pure normalization.
    #   FFN path:  relu(y@W1 + b1) = relu(hnorm @ (g1*W1) + (b1 + be1@W1))
    #   residual:  y = hnorm*g1 + be1 -> be1 joins b2's rep; g1 stays.
    #   LN2 affine (g2, be2) is applied on the host epilogue.
    g1f = np.asarray(gamma1, np.float32)
    be1f = np.asarray(beta1, np.float32)
    w1_eff = (np.asarray(W1, np.float32) * g1f[:, None])
    b1_eff = np.asarray(b1, np.float32) + be1f @ np.asarray(W1, np.float32)
    b2_eff = np.asarray(b2, np.float32) + be1f
    in_maps = []
    for c in range(N_CORES):
        h = c
        base = h * 3 * DH
        # 1/sqrt(dh) is applied via the exp's scale param (the q/k fp8
        # values stay at unit std for e4m3 precision)
        wqk_h = np.asarray(Wqkv[:, base:base + 2 * DH],
                           np.float32).astype(ml_dtypes.bfloat16)
        wv_h = np.asarray(Wqkv[:, base + 2 * DH:base + 3 * DH],
                          np.float32).astype(ml_dtypes.bfloat16)
        bqk_h = np.asarray(bqkv[base:base + 2 * DH]).reshape(-1, 1)
        bv_h = bqkv[base + 2 * DH:base + 3 * DH].reshape(-1, 1)
        tbl = pos_bias[L_MAX - 1 - (L - 1):L_MAX - 1 + L, h].astype(np.float32)
        ebv = np.exp(tbl[::-1] * SCALE).astype(ml_dtypes.bfloat16)
        ebv = np.concatenate(
            [ebv, np.ones(EBW - ebv.shape[0], ml_dtypes.bfloat16)])
        xsh = np.empty((4 * P, D), np.float32)
        for ci, (b_, ihalf) in enumerate([(0, 0), (0, 1), (1, 0), (1, 1)]):
            r0 = b_ * L + ihalf * IH + c * P
            xsh[ci * P:(ci + 1) * P] = x_flat[r0:r0 + P]
        in_maps.append({
            "xT": xT, "xsh": xsh, "wqk": wqk_h, "wv": wv_h, "wo": wo_full,
            "w1": w1_eff.astype(ml_dtypes.bfloat16),
            "w2": np.asarray(W2, np.float32).astype(ml_dtypes.bfloat16),
            "eband": ebv,
            "bqk": np.asarray(bqk_h, np.float32),
            "bv": np.asarray(bv_h, np.float32),
            "b1c": np.ascontiguousarray(b1_eff.reshape(DFF // P, P).T),
            "bo": np.asarray(bo, np.float32), "b2": b2_eff,
            "g1": g1f.astype(ml_dtypes.bfloat16),
        })
    return in_maps


def kernel(**inputs):
    if "nc" not in _cached:
        _cached["nc"] = build()
    nc = _cached["nc"]
    in_maps = _prep_inputs(**{k: np.asarray(v) for k, v in inputs.items()})
    res = bass_utils.run_bass_kernel_spmd(
        nc, in_maps, core_ids=list(range(N_CORES)),
        **_cached.get("run_kwargs", {}))
    _cached["last_result"] = res
    out = np.empty((NT, D), np.float32)
    for c in range(N_CORES):
        sh = res.results[c]["out_sh"]
        for ci, (b_, ihalf) in enumerate([(0, 0), (0, 1), (1, 0), (1, 1)]):
            r0 = b_ * L + ihalf * IH + c * P
            out[r0:r0 + P] = sh[ci * P:(ci + 1) * P]
    # LN2 affine epilogue (folded off the device critical path)
    out = out * np.asarray(inputs["gamma2"], np.float32) \
        + np.asarray(inputs["beta2"], np.float32)
    return out.reshape(B, L, D)


# revision 64
# speedup vs baseline: 1.2816x; 1.2816x over previous
"""Trainium2 Bass kernel for an attention block (MHSA with relative position
bias + 2x LayerNorm + FFN), sharded over 8 NeuronCores.

Sharding: tensor-parallel over heads for attention (core c owns head c, both
batch elements). Heads are exchanged with a bf16 AllToAll (per-head outputs,
already normalized, transposed [feat, tok]) so each core then computes the
out-projection, residual+LN1, FFN and LN2 for its own 512-token shard
locally. The host reassembles the full [2, 2048, 512] output.

v2 layout choices (vs the f32r baseline):
  - All matmul operands are bf16 (psum stays f32); weights and x are
    converted host-side, halving HBM traffic.
  - 1/sqrt(dh) is folded into Wq/bq host-side, and the relative-position
    bias is applied as exp(s)*exp(b/8): the exp runs on ACT straight from
    the scores psum, and the Toeplitz exp-band multiply runs on DVE in
    2-byte fast mode. No f32 prescale pass.
  - The exp-band is materialized once in SBUF as a [128, 4096] sliding
    window (row j' = table[j':j'+4096]); every (jt,b) tile is a reversed
    in-SBUF slice - no per-tile band DMA.
  - Softmax 1/sigma is broadcast across the 64 feature partitions with a
    rank-1 PE matmul (ones[1,64]^T @ rrow[1,1024]) instead of a DRAM
    bounce, keeping the pre-AllToAll path on-chip.
  - Phase C runs ihalf0's FFN completely (through LN2) before touching
    ihalf1, so the second AllToAll is covered by useful work.
"""
import os
import sys

for _p in ("/opt/trn_rl_repo", "/root/.axon_site/_ro/trn_rl_repo"):
    if os.path.isdir(_p) and _p not in sys.path:
        sys.path.insert(0, _p)

import numpy as np
import ml_dtypes

import concourse.bass as bass
import concourse.mybir as mybir
import concourse.tile as tile
from concourse import bacc
from concourse import bass_utils

F32 = mybir.dt.float32
F32R = mybir.dt.float32r
BF16 = mybir.dt.bfloat16
AF = mybir.ActivationFunctionType
ALU = mybir.AluOpType

N_CORES = 8
B, L, D, DFF = 2, 2048, 512, 2048
H, DH = 8, 64
L_MAX = 39000
LN_EPS = 1e-5
NT = B * L               # 4096 tokens
P = 128
IH = 1024                # i-half size inside one batch's 2048 queries
NJT = L // P             # 16 j tiles per batch
SCALE = 0.125            # 1/sqrt(dh), folded into Wq host-side
VW = DH + 2              # v block width (ones col + pad, even width)
EBW = 4352               # exp-band table length (>= 128 + 4095)
EXPC = 4.0               # exp shift: pt = exp(s/8 - EXPC) keeps the fp8
#                          weights in e4m3 range (softmax is shift-invariant)
FP8 = mybir.dt.float8e4

_cached = {}


def _ln_pair(nc, scr, h_aps, out_aps, eps_ap):
    """LayerNorm (without the affine epilogue - gamma/beta are folded on
    the host) over the free dim (512) for a PAIR of [128, 512] tiles,
    stage-batched so ACT table loads and cross-engine hops amortize."""
    n = len(h_aps)
    sq = [scr.tile([P, D], F32, tag=f"ln_sq{k}", name=f"lnsq{k}")
          for k in range(n)]
    ssum = [scr.tile([P, 1], F32, tag=f"ln_s1{k}", name=f"lns1{k}")
            for k in range(n)]
    msum = [scr.tile([P, 1], F32, tag=f"ln_s2{k}", name=f"lns2{k}")
            for k in range(n)]
    mu = [scr.tile([P, 1], F32, tag=f"ln_s3{k}", name=f"lns3{k}")
          for k in range(n)]
    var = [scr.tile([P, 1], F32, tag=f"ln_s5{k}", name=f"lns5{k}")
           for k in range(n)]
    std = [scr.tile([P, 1], F32, tag=f"ln_s6{k}", name=f"lns6{k}")
           for k in range(n)]
    rstd = [scr.tile([P, 1], F32, tag=f"ln_s7{k}", name=f"lns7{k}")
            for k in range(n)]
    nmr = [scr.tile([P, 1], F32, tag=f"ln_s8{k}", name=f"lns8{k}")
           for k in range(n)]
    # DVE-heavy: only the Sqrt crosses to ACT (2 engine hops total)
    for k in range(n):
        nc.vector.scalar_tensor_tensor(out=sq[k][:], in0=h_aps[k],
                                       scalar=1.0, in1=h_aps[k],
                                       op0=ALU.mult, op1=ALU.mult,
                                       accum_out=ssum[k][:])
    for k in range(n):
        nc.vector.tensor_reduce(out=msum[k][:], in_=h_aps[k],
                                axis=mybir.AxisListType.X, op=ALU.add)
    for k in range(n):
        nc.vector.tensor_scalar_mul(mu[k][:], msum[k][:], 1.0 / D)
        nc.vector.tensor_scalar_mul(var[k][:], ssum[k][:], 1.0 / D)
    for k in range(n):
        nc.vector.tensor_mul(msum[k][:], mu[k][:], mu[k][:])
        nc.vector.tensor_sub(var[k][:], var[k][:], msum[k][:])
    for k in range(n):
        nc.scalar.activation(std[k][:], var[k][:], AF.Sqrt, bias=eps_ap)
    for k in range(n):
        nc.vector.reciprocal(rstd[k][:], std[k][:])
        nc.vector.scalar_tensor_tensor(out=nmr[k][:], in0=mu[k][:],
                                       scalar=-1.0, in1=rstd[k][:],
                                       op0=ALU.mult, op1=ALU.mult)
    for k in range(n):
        nc.vector.tensor_scalar(out=out_aps[k], in0=h_aps[k],
                                scalar1=rstd[k][:], scalar2=nmr[k][:],
                                op0=ALU.mult, op1=ALU.add)


def build():
    nc = bacc.Bacc("TRN2", target_bir_lowering=False, debug=False,
                   num_devices=N_CORES)

    # ---- I/O ----
    xT = nc.dram_tensor("xT", [D, NT], BF16, kind="ExternalInput")
    xsh = nc.dram_tensor("xsh", [4 * P, D], F32, kind="ExternalInput")
    wqk = nc.dram_tensor("wqk", [D, 2 * DH], BF16, kind="ExternalInput")
    wv = nc.dram_tensor("wv", [D, DH], BF16, kind="ExternalInput")
    wo = nc.dram_tensor("wo", [D, D], BF16, kind="ExternalInput")
    w1 = nc.dram_tensor("w1", [D, DFF], BF16, kind="ExternalInput")
    w2 = nc.dram_tensor("w2", [DFF, D], BF16, kind="ExternalInput")
    eband = nc.dram_tensor("eband", [EBW], BF16, kind="ExternalInput")
    bqk = nc.dram_tensor("bqk", [2 * DH, 1], F32, kind="ExternalInput")
    bv = nc.dram_tensor("bv", [DH, 1], F32, kind="ExternalInput")
    b1c = nc.dram_tensor("b1c", [P, DFF // P], F32, kind="ExternalInput")
    bo = nc.dram_tensor("bo", [D], F32, kind="ExternalInput")
    b2 = nc.dram_tensor("b2", [D], F32, kind="ExternalInput")
    g1 = nc.dram_tensor("g1", [D], BF16, kind="ExternalInput")
    out_sh = nc.dram_tensor("out_sh", [4 * P, D], F32, kind="ExternalOutput")

    with tile.TileContext(nc) as tc:
        with tc.tile_pool(name="persist", bufs=1) as pers, \
             tc.tile_pool(name="phC_w", bufs=1) as pCw, \
             tc.tile_pool(name="dram", bufs=1, space="DRAM") as dram:

            # ---------- persistent SBUF (small) ----------
            ident32 = pers.tile([P, P], F32)
            identb = pers.tile([P, P], BF16)
            ident8 = pers.tile([P, P], FP8)
            bqk_sb = pers.tile([2 * DH, 1], F32)
            bv_sb = pers.tile([DH, 1], F32)
            wo_sb = [pers.tile([P, D], BF16, name=f"wo{c}") for c in range(4)]
            eps_sb = pers.tile([P, 1], F32)
            nc.vector.memset(eps_sb[:], LN_EPS)
            negc_sb = pers.tile([P, 1], F32)
            nc.vector.memset(negc_sb[:], -EXPC)
            ones_row = pers.tile([1, DH], BF16)
            nc.vector.memset(ones_row[:], 1.0)
            warm_sb = pers.tile([1, 1], F32)
            # one a2a per 128-token tile, pipelined with attention: only
            # tile 3's FFN chain remains after the last (small) collective
            oaq_sb = [[pers.tile([P, P], BF16, name=f"oa{q}_{c}")
                       for c in range(4)] for q in range(4)]
            hh0_sb = [pers.tile([P, D], F32, name=f"hh0_{m}")
                      for m in range(3)]
            x_sb = [[pers.tile([P, D], F32, name=f"x_{ih}_{b_}")
                     for b_ in range(B)] for ih in range(2)]

            from concourse.masks import make_identity
            make_identity(nc, ident32[:])
            nc.scalar.copy(identb[:], ident32[:])
            nc.scalar.copy(ident8[:], ident32[:])

            nc.sync.dma_start(bqk_sb[:], bqk.ap())
            nc.sync.dma_start(bv_sb[:], bv.ap())

            # a2a buffers: one [8*64, 128] exchange per token tile
            a2a_in = [dram.tile([N_CORES * DH, P], BF16, name=f"a2ai{i}")
                      for i in range(4)]
            a2a_out = [dram.tile([N_CORES * DH, P], BF16, name=f"a2ao{i}")
                       for i in range(4)]

            # ================= attention (phases A+B) =================
            with tc.tile_pool(name="attn_sb", bufs=1) as patt:
                # q/k in the canonical fp8 DoubleRow layout [128, 2, tok]:
                # subtile 0 rows 0..63 hold the head dim, the rest is zero
                # (matmul cost scales with output columns, not K rows)
                q8 = patt.tile([P, 2, NT], FP8)
                k8 = patt.tile([P, 2, NT], FP8)
                v_aug = patt.tile([P, 32, VW], BF16)
                outT_sb = [patt.tile([DH, L], BF16, name=f"outT{b_}")
                           for b_ in range(B)]
                # sliding-window exp-band: row j' = eband[j' : j'+4096]
                eb_all = patt.tile([P, 4096], BF16)

                # ---------- phase A: qkv projections ----------
                with tc.tile_pool(name="phA", bufs=1) as pA, \
                     tc.tile_pool(name="psA", bufs=2, space="PSUM") as psA:
                    wqk_sb = [pA.tile([P, 2 * DH], BF16, name=f"wqk{c}")
                              for c in range(4)]
                    wv_sb = [pA.tile([P, DH], BF16, name=f"wv{c}")
                             for c in range(4)]
                    for c in range(4):
                        nc.sync.dma_start(wqk_sb[c][:],
                                          wqk.ap()[c * P:(c + 1) * P, :])
                        nc.sync.dma_start(wv_sb[c][:],
                                          wv.ap()[c * P:(c + 1) * P, :])

                    vT_sb = pA.tile([DH, NT], BF16)
                    # x chunks split over the sync+ACT DMA queues; the
                    # gpsimd queue carries the no-dependency loads so the
                    # dependent kT copies never delay an x transfer
                    qdma = [nc.sync, nc.sync, nc.scalar, nc.scalar]
                    for hf in range(2):
                        xh = [pA.tile([P, L], BF16, tag=f"xh{c}",
                                      name=f"xh_{hf}_{c}", bufs=2)
                              for c in range(4)]
                        for c in range(4):
                            qdma[c].dma_start(
                                xh[c][:],
                                xT.ap()[c * P:(c + 1) * P,
                                        hf * L:(hf + 1) * L])
                        if hf == 0:
                            nc.gpsimd.dma_start(
                                eb_all[:],
                                bass.AP(eband, 0, [[1, P], [1, 4096]]))
                            # zero the unused DoubleRow K rows/subtiles
                            nc.vector.memset(q8[DH:, 0, :], 0.0)
                            nc.vector.memset(q8[:, 1, :], 0.0)
                            nc.gpsimd.memset(k8[DH:, 0, :], 0.0)
                            nc.gpsimd.memset(k8[:, 1, :], 0.0)
                            for t in range(32):
                                nc.vector.memset(v_aug[:, t, DH:VW], 1.0)
                            for ih in range(2):
                                for b_ in range(B):
                                    ci = b_ * 2 + ih
                                    nc.gpsimd.dma_start(
                                        x_sb[ih][b_][:],
                                        xsh.ap()[ci * P:(ci + 1) * P, :])
                        def v_transposes(t):
                            # transpose v^T -> v_aug natural [tok, dh];
                            # bf16 through the PE (fp8 transpose has an
                            # output-step constraint), cast to fp8 on the
                            # psum->SBUF copy. Ones cols were memset above.
                            for tt in range(4 * t, 4 * t + 4):
                                ps2 = psA.tile([P, DH], BF16, tag="vt_ps")
                                nc.tensor.transpose(
                                    ps2[:], vT_sb[:, tt * P:(tt + 1) * P],
                                    identb[:DH, :DH])
                                nc.scalar.copy(
                                    v_aug[:, tt, :DH], ps2[:])

                        for t4 in range(4):
                            t = hf * 4 + t4
                            sl = slice(t * 512, (t + 1) * 512)
                            ps = psA.tile([P, 512], F32, tag="qk_ps")
                            for c in range(4):
                                nc.tensor.matmul(
                                    ps[:], wqk_sb[c][:],
                                    xh[c][:, t4 * 512:(t4 + 1) * 512],
                                    start=(c == 0), stop=(c == 3))
                            # q|k evac on DVE (beside the v evac on ACT)
                            # into DoubleRow subtile 0, rows 0..63
                            nc.vector.tensor_scalar(
                                out=q8[:DH, 0, sl], in0=ps[:DH, :],
                                scalar1=bqk_sb[:DH, :],
                                scalar2=None, op0=ALU.add)
                            nc.vector.tensor_scalar(
                                out=k8[:DH, 0, sl], in0=ps[DH:, :],
                                scalar1=bqk_sb[DH:, :],
                                scalar2=None, op0=ALU.add)
                            psv = psA.tile([DH, 512], F32, tag="v_ps")
                            for c in range(4):
                                nc.tensor.matmul(
                                    psv[:], wv_sb[c][:],
                                    xh[c][:, t4 * 512:(t4 + 1) * 512],
                                    start=(c == 0), stop=(c == 3))
                            nc.scalar.activation(
                                vT_sb[:, t * 512:(t + 1) * 512], psv[:],
                                AF.Identity, bias=bv_sb[:])
                            # pipeline the v transposes one tile behind so
                            # they never stall the projection matmuls
                            if t > 0:
                                v_transposes(t - 1)
                        if hf == 1:
                            v_transposes(7)

                # prefetch out-proj + FFN weights now that phase A freed
                # SBUF; the sync DMA queue is otherwise idle here, and the
                # ACT queue must stay clear for the attention exps
                for c in range(4):
                    nc.sync.dma_start(wo_sb[c][:],
                                      wo.ap()[c * P:(c + 1) * P, :])
                w1_sb = [pCw.tile([P, DFF], BF16, name=f"w1_{c}")
                         for c in range(4)]
                for c in range(4):
                    nc.sync.dma_start(w1_sb[c][:],
                                      w1.ap()[c * P:(c + 1) * P, :])
                w2_sb = [pCw.tile([P, D], BF16, name=f"w2_{q}")
                         for q in range(16)]
                for q in range(16):
                    nc.sync.dma_start(w2_sb[q][:],
                                      w2.ap()[q * P:(q + 1) * P, :])
                b1_sb = pCw.tile([P, DFF // P], F32)
                nc.sync.dma_start(b1_sb[:], b1c.ap())
                reps = {}
                for nm, t, dt in (("bo", bo, F32), ("b2", b2, F32),
                                  ("g1", g1, BF16)):
                    r = pCw.tile([P, D], dt, name=f"rep_{nm}")
                    nc.gpsimd.dma_start(
                        r[:], t.ap().unsqueeze(0).broadcast_to([P, D]))
                    reps[nm] = r

                # ---------- phase B: attention, batch-outer ----------
                with tc.tile_pool(name="phB_p", bufs=4) as pP, \
                     tc.tile_pool(name="phB_r", bufs=2) as pR, \
                     tc.tile_pool(name="psB_s", bufs=3, space="PSUM") as psS, \
                     tc.tile_pool(name="psB_o", bufs=1, space="PSUM") as psO:
                    DR = mybir.MatmulPerfMode.DoubleRow
                    AV_LAG = 2   # software pipeline: AV(jt) issues after
                    #              scores(jt+AV_LAG) so PE never waits on
                    #              the exp+band-mult roundtrip
                    for b_ in range(B):
                        tb = b_ * L
                        for ihalf in range(2):
                            i0g = ihalf * IH
                            oT = psO.tile([VW, IH], F32, tag="outT_ps",
                                          name=f"oT_{b_}_{ihalf}")
                            pts = {}

                            def do_av(jt):
                                vs = v_aug[:, b_ * NJT + jt, :]
                                pt = pts.pop(jt)
                                for n2 in range(2):
                                    nc.tensor.matmul(
                                        oT[:, n2 * 512:(n2 + 1) * 512],
                                        vs, pt[:, n2 * 512:(n2 + 1) * 512],
                                        start=(jt == 0),
                                        stop=(jt == NJT - 1))

                            for jt in range(NJT):
                                # band tile: row j', col i -> eb[q0+j'+IH-1-i]
                                q0 = jt * P + IH * (1 - ihalf)
                                wrev = eb_all[:, q0 + IH - 1::-1][:, :IH]
                                sps = psS.tile([P, IH], F32, tag="s_ps")
                                for n2 in range(2):
                                    nc.tensor.matmul(
                                        sps[:, n2 * 512:(n2 + 1) * 512],
                                        k8[:, :, tb + jt * P:
                                           tb + (jt + 1) * P],
                                        q8[:, :, tb + i0g + n2 * 512:
                                           tb + i0g + (n2 + 1) * 512],
                                        start=True, stop=True,
                                        perf_mode=DR)
                                es = pP.tile([P, IH], BF16, tag="es")
                                nc.scalar.activation(es[:], sps[:], AF.Exp,
                                                     scale=SCALE,
                                                     bias=negc_sb[:])
                                pt = pP.tile([P, IH], BF16, tag="pt")
                                nc.vector.tensor_mul(pt[:], es[:], wrev)
                                pts[jt] = pt
                                if jt >= AV_LAG:
                                    do_av(jt - AV_LAG)
                            for jt in range(NJT - AV_LAG, NJT):
                                do_av(jt)
                            if b_ == 1 and ihalf == 1:
                                # prewarm ACT Sqrt table so LN1 doesn't pay
                                # the table reload on the post-attention path
                                nc.scalar.activation(warm_sb[:],
                                                     eps_sb[:1, :1],
                                                     AF.Sqrt, bias=0.0)
                            # evict: normalize by 1/sigma; sigma row is
                            # broadcast over the 64 feature partitions with
                            # a rank-1 PE matmul (no DRAM bounce)
                            sl = slice(i0g, i0g + IH)
                            srow = pR.tile([1, IH], F32, tag="srow")
                            nc.vector.tensor_copy(srow[:],
                                                  oT[DH:DH + 1, :])
                            rrow = pR.tile([1, IH], F32, tag="rrow")
                            rscr = pR.tile([1, IH], F32, tag="rscr")
                            nc.vector.reciprocal_approx_accurate(
                                rrow[:], srow[:], rscr[:])
                            rrowb = pR.tile([1, IH], BF16, tag="rrowb")
                            nc.vector.tensor_copy(rrowb[:], rrow[:])
                            rrep = psS.tile([DH, IH], F32, tag="s_ps",
                                            name=f"rrep_{b_}_{ihalf}")
                            for n2 in range(2):
                                nc.tensor.matmul(
                                    rrep[:, n2 * 512:(n2 + 1) * 512],
                                    ones_row[:],
                                    rrowb[:, n2 * 512:(n2 + 1) * 512],
                                    start=True, stop=True)
                            rreps = pR.tile([DH, IH], F32, tag="rreps")
                            nc.vector.tensor_copy(rreps[:], rrep[:])
                            nc.vector.tensor_mul(outT_sb[b_][:, sl],
                                                 oT[:DH, :], rreps[:])
                            # ship this chunk's pieces and exchange heads
                            # immediately (q3 = token-tile index b*2+ihalf);
                            # sync+gpsimd queues: ACT stays clear for exps
                            q3 = b_ * 2 + ihalf
                            for cd in range(N_CORES):
                                eng = nc.sync if cd % 2 == 0 else nc.gpsimd
                                eng.dma_start(
                                    a2a_in[q3][cd * DH:(cd + 1) * DH, :],
                                    outT_sb[b_][:, i0g + cd * P:
                                                i0g + (cd + 1) * P])
                            nc.gpsimd.collective_compute(
                                "AllToAll", ALU.bypass,
                                replica_groups=[list(range(N_CORES))],
                                ins=[a2a_in[q3][:].opt()],
                                outs=[a2a_out[q3][:].opt()])
                            for c in range(4):
                                nc.gpsimd.dma_start(
                                    oaq_sb[q3][c][:],
                                    a2a_out[q3][c * P:(c + 1) * P, :])

                    # hoisted: out-proj + residual for a2a#1's three token
                    # tiles runs in the attention psum pool (s_ps tag) right
                    # at attention end, avoiding cross-pool bank-alias waits
                    for m in range(3):
                        pop0 = psS.tile([P, D], F32, tag="s_ps",
                                        name=f"pop0_{m}")
                        for c in range(4):
                            nc.tensor.matmul(
                                pop0[:, :D],
                                oaq_sb[m][c][:],
                                wo_sb[c][:], start=(c == 0), stop=(c == 3))
                        nc.vector.scalar_tensor_tensor(
                            out=hh0_sb[m][:], in0=pop0[:, :D], scalar=1.0,
                            in1=x_sb[m % 2][m // 2][:], op0=ALU.mult,
                            op1=ALU.add)
                        nc.vector.tensor_add(hh0_sb[m][:], hh0_sb[m][:],
                                             reps["bo"][:])

            # ---------- phase C: out-proj + residual + LN1 + FFN + LN2 ----
            # batch0's full chain runs first so it covers the a2a#2 wait
            with tc.tile_pool(name="phC", bufs=2) as pC, \
                 tc.tile_pool(name="phC_scr", bufs=2) as scr, \
                 tc.tile_pool(name="psC", bufs=2, space="PSUM") as psC, \
                 tc.tile_pool(name="psC2", bufs=2, space="PSUM") as psC2, \
                 tc.tile_pool(name="psC3", bufs=2, space="PSUM") as psC3:
                # group 0 = token tiles 0..2 (a2a#1), group 1 = tile 3 (#2)
                GT = {0: [0, 1, 2], 1: [3]}
                hgs = {}
                hgTs = {}
                f1Ts = {}

                def prep(g):
                    tl = GT[g]
                    # LN1 output directly in bf16 (matmul + residual path)
                    hgb = [pC.tile([P, D], BF16, tag="hgb", bufs=5,
                                   name=f"hgb_{ci}") for ci in tl]
                    hgs[g] = hgb
                    if g == 0:
                        hhs = hh0_sb
                    else:
                        hhs = []
                        for ci in tl:
                            pop = psC3.tile([P, D], F32, tag="po_ps")
                            for c in range(4):
                                nc.tensor.matmul(
                                    pop[:], oaq_sb[ci][c][:],
                                    wo_sb[c][:], start=(c == 0),
                                    stop=(c == 3))
                            hh = pC.tile([P, D], F32, tag="hh", bufs=4,
                                         name=f"hh_{ci}")
                            nc.vector.scalar_tensor_tensor(
                                out=hh[:], in0=pop[:], scalar=1.0,
                                in1=x_sb[ci % 2][ci // 2][:], op0=ALU.mult,
                                op1=ALU.add)
                            nc.vector.tensor_add(hh[:], hh[:],
                                                 reps["bo"][:])
                            hhs.append(hh)
                    _ln_pair(nc, scr, [t[:] for t in hhs],
                             [t[:] for t in hgb], eps_sb[:])

                def transposes(g):
                    tl = GT[g]
                    W = len(tl) * P
                    hgT = pC.tile([P, 4 * W], BF16, tag=f"hgT{g}", bufs=1,
                                  name=f"hgT_{g}")
                    hgTs[g] = (hgT, W)
                    for m in range(len(tl)):
                        for c in range(4):
                            tps = psC.tile([P, P], BF16, tag="tr_ps")
                            nc.tensor.transpose(
                                tps[:], hgs[g][m][:, c * P:(c + 1) * P],
                                identb[:])
                            nc.scalar.copy(
                                hgT[:, c * W + m * P:c * W + (m + 1) * P],
                                tps[:])

                def ffn1(g):
                    hgT, W = hgTs[g]
                    f1T = pC.tile([P, 16 * W], BF16, tag=f"f1T{g}", bufs=1,
                                  name=f"f1T_{g}")
                    f1Ts[g] = (f1T, W)
                    # batch TB dff-tiles per psum tile so narrow-W groups
                    # don't pay per-matmul/relu overheads 16 times
                    TB = max(1, 512 // W)
                    for t0 in range(0, 16, TB):
                        fps = psC.tile([P, TB * W], F32, tag="f1_ps")
                        for tq in range(TB):
                            t = t0 + tq
                            for c in range(4):
                                nc.tensor.matmul(
                                    fps[:, tq * W:(tq + 1) * W],
                                    w1_sb[c][:, t * P:(t + 1) * P],
                                    hgT[:, c * W:(c + 1) * W],
                                    start=(c == 0), stop=(c == 3))
                        if TB == 1:
                            nc.scalar.activation(
                                f1T[:, t0 * W:(t0 + TB) * W], fps[:],
                                AF.Relu, bias=b1_sb[:, t0:t0 + 1])
                        else:
                            # b1 varies per dff-tile: bias via per-tile AP
                            for tq in range(TB):
                                t = t0 + tq
                                nc.scalar.activation(
                                    f1T[:, t * W:(t + 1) * W],
                                    fps[:, tq * W:(tq + 1) * W],
                                    AF.Relu, bias=b1_sb[:, t:t + 1])

                def ffn2_out(g):
                    tl = GT[g]
                    f1T, W = f1Ts[g]
                    zzs = []
                    for m, ci in enumerate(tl):
                        ops = psC2.tile([P, D], F32, tag="f2_ps")
                        for q in range(16):
                            nc.tensor.matmul(
                                ops[:],
                                f1T[:, q * W + m * P:q * W + (m + 1) * P],
                                w2_sb[q][:], start=(q == 0), stop=(q == 15))
                        # LN2 residual: hnorm1*g1 (be1 is folded into b2)
                        hgr = pC.tile([P, D], BF16, tag="hgr", bufs=4,
                                      name=f"hgr_{ci}")
                        nc.vector.tensor_mul(hgr[:], hgs[g][m][:],
                                             reps["g1"][:])
                        zz = pC.tile([P, D], F32, tag="zz", bufs=4,
                                     name=f"zz_{ci}")
                        nc.vector.tensor_add(zz[:], ops[:], hgr[:])
                        nc.vector.tensor_add(zz[:], zz[:], reps["b2"][:])
                        zzs.append(zz)
                    yys = [pC.tile([P, D], F32, tag="yy", bufs=4,
                                   name=f"yy_{ci}") for ci in tl]
                    _ln_pair(nc, scr, [t[:] for t in zzs],
                             [t[:] for t in yys], eps_sb[:])
                    for m, ci in enumerate(tl):
                        nc.sync.dma_start(out_sh.ap()[ci * P:(ci + 1) * P, :],
                                          yys[m][:])

                prep(0)
                transposes(0)
                ffn1(0)
                ffn2_out(0)
                prep(1)
                transposes(1)
                ffn1(1)
                ffn2_out(1)
    nc.compile()
    return nc


def _prep_inputs(x, Wqkv, bqkv, Wo, bo, pos_bias, W1, b1, W2, b2,
                 gamma1, beta1, gamma2, beta2):
    x_flat = np.ascontiguousarray(x.reshape(NT, D), dtype=np.float32)
    xT = np.ascontiguousarray(x_flat.T).astype(ml_dtypes.bfloat16)
    wo_full = np.asarray(Wo, np.float32).astype(ml_dtypes.bfloat16)
    # LN affine folds: y = LN(h)*g + b with LN the pure normalization.
    #   FFN path:  relu(y@W1 + b1) = relu(hnorm @ (g1*W1) + (b1 + be1@W1))
    #   residual:  y = hnorm*g1 + be1 -> be1 joins b2's rep; g1 stays.
    #   LN2 affine (g2, be2) is applied on the host epilogue.
    g1f = np.asarray(gamma1, np.float32)
    be1f = np.asarray(beta1, np.float32)
    w1_eff = (np.asarray(W1, np.float32) * g1f[:, None])
    b1_eff = np.asarray(b1, np.float32) + be1f @ np.asarray(W1, np.float32)
    b2_eff = np.asarray(b2, np.float32) + be1f
    in_maps = []
    for c in range(N_CORES):
        h = c
        base = h * 3 * DH
        # 1/sqrt(dh) is applied via the exp's scale param (the q/k fp8
        # values stay at unit std for e4m3 precision)
        wqk_h = np.asarray(Wqkv[:, base:base + 2 * DH],
                           np.float32).astype(ml_dtypes.bfloat16)
        wv_h = np.asarray(Wqkv[:, base + 2 * DH:base + 3 * DH],
                          np.float32).astype(ml_dtypes.bfloat16)
        bqk_h = np.asarray(bqkv[base:base + 2 * DH]).reshape(-1, 1)
        bv_h = bqkv[base + 2 * DH:base + 3 * DH].reshape(-1, 1)
        tbl = pos_bias[L_MAX - 1 - (L - 1):L_MAX - 1 + L, h].astype(np.float32)
        ebv = np.exp(tbl[::-1] * SCALE).astype(ml_dtypes.bfloat16)
        ebv = np.concatenate(
            [ebv, np.ones(EBW - ebv.shape[0], ml_dtypes.bfloat16)])
        xsh = np.empty((4 * P, D), np.float32)
        for ci, (b_, ihalf) in enumerate([(0, 0), (0, 1), (1, 0), (1, 1)]):
            r0 = b_ * L + ihalf * IH + c * P
            xsh[ci * P:(ci + 1) * P] = x_flat[r0:r0 + P]
        in_maps.append({
            "xT": xT, "xsh": xsh, "wqk": wqk_h, "wv": wv_h, "wo": wo_full,
            "w1": w1_eff.astype(ml_dtypes.bfloat16),
            "w2": np.asarray(W2, np.float32).astype(ml_dtypes.bfloat16),
            "eband": ebv,
            "bqk": np.asarray(bqk_h, np.float32),
            "bv": np.asarray(bv_h, np.float32),
            "b1c": np.ascontiguousarray(b1_eff.reshape(DFF // P, P).T),
            "bo": np.asarray(bo, np.float32), "b2": b2_eff,
            "g1": g1f.astype(ml_dtypes.bfloat16),
        })
    return in_maps


def kernel(**inputs):
    if "nc" not in _cached:
        _cached["nc"] = build()
    nc = _cached["nc"]
    in_maps = _prep_inputs(**{k: np.asarray(v) for k, v in inputs.items()})
    res = bass_utils.run_bass_kernel_spmd(
        nc, in_maps, core_ids=list(range(N_CORES)),
        **_cached.get("run_kwargs", {}))
    _cached["last_result"] = res
    out = np.empty((NT, D), np.float32)
    for c in range(N_CORES):
        sh = res.results[c]["out_sh"]
        for ci, (b_, ihalf) in enumerate([(0, 0), (0, 1), (1, 0), (1, 1)]):
            r0 = b_ * L + ihalf * IH + c * P
            out[r0:r0 + P] = sh[ci * P:(ci + 1) * P]
    # LN2 affine epilogue (folded off the device critical path)
    out = out * np.asarray(inputs["gamma2"], np.float32) \
        + np.asarray(inputs["beta2"], np.float32)
    return out.reshape(B, L, D)
